# revision 12
# baseline (speedup 1.0000x reference)
"""BlockSparseMLP (MoE top-2 routing, 8 experts) — Trainium2 Bass kernel.

Strategy: pairwise expert-tensor-parallelism for load balance.  The
router (host) yields per-expert token counts n_e with max ~546 but mean
512; pure expert-parallel paces all 8 cores at the heaviest expert.
Instead experts are paired heavy+light (greedy: i-th largest with i-th
smallest) and each pair is served by TWO cores, each holding HALF of the
F dimension of BOTH experts' weights (same weight bytes per core as
expert-parallel).  Both cores process all of the pair's tokens on their
F-half; the down-projection is then a partial sum over F, and the host
adds the two cores' partials during the combine (free — no device
collective).  Per-core token slots: [0, capA) heavy expert, [capA,
capA+capB) light expert, capA = max heavy count, capB = max light
count; capA+capB ~ 1056 vs 2*546 = 1092 worth of slot-work for pure
expert-parallel.

Device program per core (SPMD, shapes uniform):
   f-tiles 0..21  = heavy expert's F-half   (tokens [0, capA))
   f-tiles 22..43 = light expert's F-half   (tokens [capA, capA+capB))
   gT = Wg.T @ xT ; uT = Wu.T @ xT ; aT = silu(gT)*uT   (bf16)
   dT_partial = Wd.T @ aT                                (bf16 out)

All inputs are pre-cast to bf16 on the host (HBM read ~74 MB/core) and
pre-swizzled into per-DMA-block partition-major layout.  PSUM: six
exact-size accumulators (A-chunk0 290, A-chunk1 256, B-chunk 510 for
each of g/u) shared by phase 2 (ds=0 reuses the g tags, ds=1 the u
tags) + 1 warm-up bank.
"""

import os

import ml_dtypes
import numpy as np

T, D, F, E, TOPK = 2048, 2048, 5632, 8, 2
P = 128
KD = D // P      # 16 k-subtiles over D
KF = F // P      # 44 f-tiles total (22 per expert F-half)
KFH = KF // 2    # 22
FG = 4           # f-tiles per phase-1 weight DMA block
NFG = KF // FG   # 11 phase-1 blocks
DG = 2           # d-tiles per phase-2 psum group (256 D columns)
NDG = KD // DG   # 8 phase-2 d-groups
KO2 = 4          # f-subtiles per phase-2 weight DMA block
NFB = KF // KO2  # 11 phase-2 blocks per d-group

BF16 = ml_dtypes.bfloat16

_COMPILED = {}   # (capA, capB) -> nc
LAST_RESULT = None  # BassKernelResults of the most recent run (for test.py)


def _chunks(cap):
    """Split cap into moving-dim chunks: single if <=512, else pieces in
    [256, 512] (>=256 keeps LDWEIGHTS hidden under the matmul)."""
    assert cap % 2 == 0
    if cap <= 512:
        return [cap]
    n512, rem = divmod(cap, 512)
    if rem == 0:
        return [512] * n512
    if rem >= 256:
        return [512] * n512 + [rem]
    return [512] * (n512 - 1) + [256 + rem, 256]


def _build(capA, capB):
    """Build + compile the SPMD Tile program."""
    import concourse.bass as bass  # noqa: F401
    import concourse.mybir as mybir
    import concourse.tile as tile
    from concourse import bacc

    f32 = mybir.dt.float32
    bf16 = mybir.dt.bfloat16
    mult = mybir.AluOpType.mult

    cap = capA + capB
    # region r: (first f-tile, first token slot, chunk widths)
    regs = [(0, 0, _chunks(capA)), (KFH, capA, _chunks(capB))]

    def reg_of(ft):
        return regs[0] if ft < KFH else regs[1]

    nc = bacc.Bacc("TRN2", target_bir_lowering=False, debug=False,
                   enable_asserts=False, num_devices=E)

    xt_d = nc.dram_tensor("xt", [P, KD, cap], bf16, kind="ExternalInput").ap()
    wg_d = nc.dram_tensor("wg", [NFG, P, KD, P * FG], bf16,
                          kind="ExternalInput").ap()
    wu_d = nc.dram_tensor("wu", [NFG, P, KD, P * FG], bf16,
                          kind="ExternalInput").ap()
    wd_d = nc.dram_tensor("wd", [NDG, NFB, P, KO2, P * DG], bf16,
                          kind="ExternalInput").ap()
    out_d = nc.dram_tensor("out_t", [D, cap], bf16, kind="ExternalOutput").ap()
    scr_d = nc.dram_tensor("scr", [P, 512], f32).ap()   # warm-up sink

    with tile.TileContext(nc) as tc:
        with (
            tc.tile_pool(name="resident", bufs=1) as rpool,
            tc.tile_pool(name="w1", bufs=3) as w1pool,
            tc.tile_pool(name="wd2", bufs=4) as wd2pool,
            tc.tile_pool(name="outp", bufs=4) as outpool,
            tc.tile_pool(name="psum", bufs=1, space="PSUM") as ppool,
        ):
            xt = rpool.tile([P, KD, cap], bf16)
            # a: A f-tile i in cols [0, capA) of plane i, B f-tile i in
            # cols [capA, cap) of plane i — packed, 22 planes not 44.
            at = rpool.tile([P, KFH, cap], bf16)

            def psum(kind, reg_idx, ci, cn, name=None):
                return ppool.tile([P, cn], f32,
                                  tag=f"{kind}{'AB'[reg_idx]}{ci}", name=name)

            # Warm-up: PE HAM clock ramp (1.2 -> 2.4 GHz) while first DMAs
            # are in flight; sized to end right when the first weight +
            # token DMAs complete so the clock never dips back to half
            # rate (an idle PE in the first ~30us downclocks to k=4 and
            # costs ~5us of half-rate work).
            NWARM = 23
            warm = rpool.tile([P, 512], bf16)
            nc.vector.memset(warm[:], 0.0)
            wps = ppool.tile([P, 512], f32, tag="warm", name="warm_ps")
            for i in range(NWARM):
                nc.tensor.matmul(wps[:], warm[:, :P], warm[:],
                                 start=(i == 0), stop=(i == NWARM - 1))
            wout = rpool.tile([P, 512], f32)
            nc.vector.tensor_copy(out=wout[:], in_=wps[:])
            nc.sync.dma_start(scr_d[:], wout[:])

            for fg in range(NFG):
                wgb = w1pool.tile([P, KD, P * FG], bf16, tag="wgb",
                                  name=f"wgb_{fg}")
                wub = w1pool.tile([P, KD, P * FG], bf16, tag="wub",
                                  name=f"wub_{fg}")
                if fg == 0:
                    # Queue order on the SWDGE ring decides arrival order
                    # (sync is NOT usable here: its first instruction gates
                    # on the warm-up, stalling anything queued behind it).
                    # The first f-tile's 16 k-passes consume ALL of xt, so
                    # xt streams right after the first 128-col wg slice, in
                    # 4-ktile pieces (fewer per-DMA issue latencies).
                    nc.gpsimd.dma_start(wgb[:, :, :P], wg_d[0][:, :, :P])
                    for k0 in range(0, KD, 4):
                        nc.gpsimd.dma_start(xt[:, k0:k0 + 4, :],
                                            xt_d[:, k0:k0 + 4, :])
                    nc.gpsimd.dma_start(wub[:, :, :P], wu_d[0][:, :, :P])
                    for s in range(1, FG):
                        sl = slice(s * P, (s + 1) * P)
                        nc.gpsimd.dma_start(wgb[:, :, sl], wg_d[0][:, :, sl])
                        nc.gpsimd.dma_start(wub[:, :, sl], wu_d[0][:, :, sl])
                else:
                    kh = KD // 2
                    nc.gpsimd.dma_start(wgb[:, :kh, :], wg_d[fg][:, :kh, :])
                    nc.gpsimd.dma_start(wgb[:, kh:, :], wg_d[fg][:, kh:, :])
                    nc.gpsimd.dma_start(wub[:, :kh, :], wu_d[fg][:, :kh, :])
                    nc.gpsimd.dma_start(wub[:, kh:, :], wu_d[fg][:, kh:, :])

                # ---- phase 1: gT/uT = W.T @ xT, aT = silu(gT)*uT ----
                for fs in range(FG):
                    ft = fg * FG + fs
                    f0, t0, cws = reg_of(ft)
                    ridx = 0 if ft < KFH else 1
                    if ft == 0:
                        # xt pieces land ~1.4us apart while this tile wants
                        # one 2-ko piece per ~0.5us: interleave both chunks
                        # per ko and pad each inter-piece hole with filler
                        # matmuls (no deps) so the PE stays busy and HAM
                        # never drops to half clock.
                        pgs = [psum("g", 0, ci, cn, name=f"pg0_{ci}")
                               for ci, cn in enumerate(cws)]
                        pus = [psum("u", 0, ci, cn, name=f"pu0_{ci}")
                               for ci, cn in enumerate(cws)]
                        for ko in range(KD):
                            c0 = t0
                            for ci, cn in enumerate(cws):
                                nc.tensor.matmul(
                                    pgs[ci][:], wgb[:, ko, :P],
                                    xt[:, ko, c0:c0 + cn],
                                    start=(ko == 0), stop=(ko == KD - 1))
                                c0 += cn
                            if ko % 4 == 3 and ko < KD - 4:
                                for _ in range(5):
                                    nc.tensor.matmul(wps[:], warm[:, :P],
                                                     warm[:], start=True,
                                                     stop=True)
                        for ko in range(KD):
                            c0 = t0
                            for ci, cn in enumerate(cws):
                                nc.tensor.matmul(
                                    pus[ci][:], wub[:, ko, :P],
                                    xt[:, ko, c0:c0 + cn],
                                    start=(ko == 0), stop=(ko == KD - 1))
                                c0 += cn
                        c0 = t0
                        for ci, cn in enumerate(cws):
                            a_sl = at[:, ft - f0, c0:c0 + cn]
                            nc.scalar.activation(
                                a_sl, pgs[ci][:],
                                mybir.ActivationFunctionType.Silu)
                            nc.vector.tensor_tensor(a_sl, a_sl, pus[ci][:],
                                                    mult)
                            c0 += cn
                        continue
                    c0 = t0
                    for ci, cn in enumerate(cws):
                        pg = psum("g", ridx, ci, cn, name=f"pg_{ft}_{ci}")
                        pu = psum("u", ridx, ci, cn, name=f"pu_{ft}_{ci}")
                        for ko in range(KD):
                            nc.tensor.matmul(
                                pg[:], wgb[:, ko, fs * P:(fs + 1) * P],
                                xt[:, ko, c0:c0 + cn],
                                start=(ko == 0), stop=(ko == KD - 1))
                        for ko in range(KD):
                            nc.tensor.matmul(
                                pu[:], wub[:, ko, fs * P:(fs + 1) * P],
                                xt[:, ko, c0:c0 + cn],
                                start=(ko == 0), stop=(ko == KD - 1))
                        a_sl = at[:, ft - f0, c0:c0 + cn]
                        nc.scalar.activation(
                            a_sl, pg[:], mybir.ActivationFunctionType.Silu)
                        nc.vector.tensor_tensor(a_sl, a_sl, pu[:], mult)
                        c0 += cn

            # ---- phase 2: dT_partial = Wd.T @ aT (combine on host) ----
            for dg in range(NDG):
                # accumulators: [ds][region][chunk]; ds=0 reuses g tags,
                # ds=1 the u tags (exact same widths)
                pds = [[[psum("gu"[ds], ridx, ci, cn,
                              name=f"pd_{dg}_{ds}_{ridx}_{ci}")
                         for ci, cn in enumerate(regs[ridx][2])]
                        for ridx in range(2)]
                       for ds in range(DG)]
                # last d-group runs B's blocks first so the kernel's very
                # last psum->sbuf copy + out DMA is the A part — a shorter
                # serial tail after the final matmul.  Accumulation flags
                # follow processing order (per-region counters), not fk.
                last = dg == NDG - 1
                fbseq = ([6, 7, 8, 9, 10, 5, 0, 1, 2, 3, 4] if last
                         else list(range(NFB)))
                done = [0, 0]
                for fb in fbseq:
                    wdb = wd2pool.tile([P, KO2, P * DG], bf16, tag="wdb",
                                       name=f"wdb_{dg}_{fb}")
                    kh2 = KO2 // 2
                    nc.gpsimd.dma_start(wdb[:, :kh2, :],
                                        wd_d[dg, fb][:, :kh2, :])
                    nc.gpsimd.dma_start(wdb[:, kh2:, :],
                                        wd_d[dg, fb][:, kh2:, :])
                    for ko in range(KO2):
                        fk = fb * KO2 + ko
                        f0, t0, cws = reg_of(fk)
                        ridx = 0 if fk < KFH else 1
                        for ds in range(DG):
                            c0 = t0
                            for ci, cn in enumerate(cws):
                                nc.tensor.matmul(
                                    pds[ds][ridx][ci][:],
                                    wdb[:, ko, ds * P:(ds + 1) * P],
                                    at[:, fk - f0, c0:c0 + cn],
                                    start=(done[ridx] == 0),
                                    stop=(done[ridx] == KFH - 1))
                                c0 += cn
                        done[ridx] += 1
                for ds in range(DG):
                    ot = outpool.tile([P, cap], bf16, tag="ot")
                    dt_idx = dg * DG + ds
                    orow = out_d[dt_idx * P:(dt_idx + 1) * P, :]
                    # emit each region's slice as soon as its accumulation
                    # is done (the other region's matmuls overlap the copy)
                    rseq = (1, 0) if last else (0, 1)
                    for ridx in rseq:
                        f0, t0, cws = regs[ridx]
                        c0 = t0
                        for ci, cn in enumerate(cws):
                            nc.vector.tensor_copy(out=ot[:, c0:c0 + cn],
                                                  in_=pds[ds][ridx][ci][:])
                            c0 += cn
                        nc.sync.dma_start(orow[:, t0:c0], ot[:, t0:c0])

    nc.compile()
    return nc


def _swizzle_w1(w):
    """[D, F] -> [NFG, P, KD, P*FG] block-major, partition-contiguous."""
    return np.ascontiguousarray(
        w.reshape(KD, P, NFG, P * FG).transpose(2, 1, 0, 3)).astype(BF16)


def _swizzle_wd(w):
    """[F, D] -> [NDG, NFB, P, KO2, P*DG] block-major."""
    return np.ascontiguousarray(
        w.reshape(NFB, KO2, P, NDG, P * DG).transpose(3, 0, 2, 1, 4)).astype(BF16)


def kernel(x, gate_tensor, Wg, Wu, Wd):
    global LAST_RESULT
    from concourse.bass_interp import get_hw_module
    from concourse.bass_utils import run_bass_kernel_spmd

    x = np.ascontiguousarray(np.asarray(x, dtype=np.float32))
    gate_tensor = np.asarray(gate_tensor, dtype=np.float32)
    Wg = np.asarray(Wg, dtype=np.float32)
    Wu = np.asarray(Wu, dtype=np.float32)
    Wd = np.asarray(Wd, dtype=np.float32)

    # ---- router (replicated; tiny: T*D*E flops) ----
    logits = x @ gate_tensor                      # [T, E] fp32
    m = logits.max(axis=-1, keepdims=True)
    p = np.exp(logits - m, dtype=np.float32)
    p /= p.sum(axis=-1, keepdims=True)
    topi = np.argsort(-p, axis=-1, kind="stable")[:, :TOPK]      # [T, K]
    topw = np.take_along_axis(p, topi, axis=-1)
    topw = topw / (topw.sum(axis=-1, keepdims=True) + 1e-20)

    idx = []          # tokens routed to each expert
    wts = []          # their combine weights
    for e in range(E):
        sel = (topi == e)                         # [T, K]; <=1 True per row
        idx.append(np.nonzero(sel.any(axis=-1))[0])
        wts.append(topw[sel].astype(np.float32))  # row-major == token order

    # ---- pairing: i-th heaviest with i-th lightest ----
    counts = np.array([len(t) for t in idx])
    order = np.argsort(-counts, kind="stable")
    pairs = [(int(order[i]), int(order[E - 1 - i])) for i in range(E // 2)]
    capA = max(2, (int(counts[order[:E // 2]].max()) + 1) // 2 * 2)
    capB = max(2, (int(counts[order[E // 2:]].max()) + 1) // 2 * 2)
    cap = capA + capB

    key = (capA, capB)
    if key not in _COMPILED:
        _COMPILED[key] = _build(capA, capB)
    nc = _COMPILED[key]

    # ---- dispatch: per-core inputs (pre-swizzled, bf16) ----
    halfF = F // 2
    in_maps = []
    for p_i, (eh, el) in enumerate(pairs):
        nh, nl = len(idx[eh]), len(idx[el])
        xt = np.zeros((P, KD, cap), dtype=BF16)
        xt[:, :, :nh] = (x[idx[eh]].T.reshape(KD, P, nh)
                         .transpose(1, 0, 2).astype(BF16))
        xt[:, :, capA:capA + nl] = (x[idx[el]].T.reshape(KD, P, nl)
                                    .transpose(1, 0, 2).astype(BF16))
        for h in range(2):
            hsl = slice(h * halfF, (h + 1) * halfF)
            wg = _swizzle_w1(np.concatenate(
                [Wg[eh][:, hsl], Wg[el][:, hsl]], axis=1))
            wu = _swizzle_w1(np.concatenate(
                [Wu[eh][:, hsl], Wu[el][:, hsl]], axis=1))
            wd = _swizzle_wd(np.concatenate(
                [Wd[eh][hsl, :], Wd[el][hsl, :]], axis=0))
            in_maps.append({"xt": xt, "wg": wg, "wu": wu, "wd": wd})

    trace = bool(int(os.environ.get("KERNEL_TRACE", "0")))
    old_m = nc.m
    nc.m = get_hw_module(nc.m)
    try:
        try:
            res = run_bass_kernel_spmd(nc, in_maps, core_ids=list(range(E)),
                                       trace=trace)
        except (ImportError, ModuleNotFoundError):
            # tracing requested but this image lacks the axon NTFF profile
            # hook -- rerun without tracing
            os.environ["BASS_NEVER_TRACE"] = "1"
            res = run_bass_kernel_spmd(nc, in_maps, core_ids=list(range(E)),
                                       trace=False)
    finally:
        nc.m = old_m
    LAST_RESULT = res

    # ---- combine: add F-half partials, weight, scatter-add ----
    out = np.zeros((T, D), dtype=np.float32)
    for p_i, (eh, el) in enumerate(pairs):
        nh, nl = len(idx[eh]), len(idx[el])
        d = (res.results[2 * p_i]["out_t"].astype(np.float32)
             + res.results[2 * p_i + 1]["out_t"].astype(np.float32))
        out[idx[eh]] += wts[eh][:, None] * d[:, :nh].T
        out[idx[el]] += wts[el][:, None] * d[:, capA:capA + nl].T
    return out


# revision 13
# speedup vs baseline: 1.1979x; 1.1979x over previous
"""BlockSparseMLP (MoE top-2 routing, 8 experts) — Trainium2 Bass kernel.

Strategy: pairwise expert-tensor-parallelism for load balance.  The
router (host) yields per-expert token counts n_e with max ~546 but mean
512; pure expert-parallel paces all 8 cores at the heaviest expert.
Instead experts are paired heavy+light (greedy: i-th largest with i-th
smallest) and each pair is served by TWO cores, each holding HALF of the
F dimension of BOTH experts' weights (same weight bytes per core as
expert-parallel).  Both cores process all of the pair's tokens on their
F-half; the down-projection is then a partial sum over F, and the host
adds the two cores' partials during the combine (free — no device
collective).  Per-core token slots: [0, capA) heavy expert, [capA,
capA+capB) light expert, capA = max heavy count, capB = max light
count; capA+capB ~ 1056 vs 2*546 = 1092 worth of slot-work for pure
expert-parallel.

Device program per core (SPMD, shapes uniform):
   f-tiles 0..21  = heavy expert's F-half   (tokens [0, capA))
   f-tiles 22..43 = light expert's F-half   (tokens [capA, capA+capB))
   gT = Wg.T @ xT ; uT = Wu.T @ xT ; aT = silu(gT)*uT   (bf16)
   dT_partial = Wd.T @ aT                                (bf16 out)

All inputs are pre-cast to bf16 on the host (HBM read ~74 MB/core) and
pre-swizzled into per-DMA-block partition-major layout.  PSUM: six
exact-size accumulators (A-chunk0 290, A-chunk1 256, B-chunk 510 for
each of g/u) shared by phase 2 (ds=0 reuses the g tags, ds=1 the u
tags) + 1 warm-up bank.
"""

import os

import ml_dtypes
import numpy as np

T, D, F, E, TOPK = 2048, 2048, 5632, 8, 2
P = 128
KD = D // P      # 16 k-subtiles over D
KF = F // P      # 44 f-tiles total (22 per expert F-half)
KFH = KF // 2    # 22
FG = 4           # f-tiles per phase-1 weight DMA block
NFG = KF // FG   # 11 phase-1 blocks
DG = 2           # d-tiles per phase-2 psum group (256 D columns)
NDG = KD // DG   # 8 phase-2 d-groups
KO2 = 4          # f-subtiles per phase-2 weight DMA block
NFB = KF // KO2  # 11 phase-2 blocks per d-group

BF16 = ml_dtypes.bfloat16

_COMPILED = {}   # (capA, capB) -> nc
LAST_RESULT = None  # BassKernelResults of the most recent run (for test.py)


def _chunks(cap):
    """Split cap into moving-dim chunks: single if <=512, else pieces in
    [256, 512] (>=256 keeps LDWEIGHTS hidden under the matmul)."""
    assert cap % 2 == 0
    if cap <= 512:
        return [cap]
    n512, rem = divmod(cap, 512)
    if rem == 0:
        return [512] * n512
    if rem >= 256:
        return [512] * n512 + [rem]
    return [512] * (n512 - 1) + [256 + rem, 256]


def _build(capA, capB):
    """Build + compile the SPMD Tile program."""
    import concourse.bass as bass  # noqa: F401
    import concourse.mybir as mybir
    import concourse.tile as tile
    from concourse import bacc

    f32 = mybir.dt.float32
    bf16 = mybir.dt.bfloat16
    mult = mybir.AluOpType.mult

    cap = capA + capB
    # region r: (first f-tile, first token slot, chunk widths)
    regs = [(0, 0, _chunks(capA)), (KFH, capA, _chunks(capB))]

    def reg_of(ft):
        return regs[0] if ft < KFH else regs[1]

    nc = bacc.Bacc("TRN2", target_bir_lowering=False, debug=False,
                   enable_asserts=False, num_devices=E)

    xt_d = nc.dram_tensor("xt", [P, KD, cap], bf16, kind="ExternalInput").ap()
    wg_d = nc.dram_tensor("wg", [NFG, P, FG, KD, P], bf16,
                          kind="ExternalInput").ap()
    wu_d = nc.dram_tensor("wu", [NFG, P, FG, KD, P], bf16,
                          kind="ExternalInput").ap()
    wd_d = nc.dram_tensor("wd", [NDG, NFB, P, KO2, P * DG], bf16,
                          kind="ExternalInput").ap()
    out_d = nc.dram_tensor("out_t", [D, cap], bf16, kind="ExternalOutput").ap()
    scr_d = nc.dram_tensor("scr", [P, 512], f32).ap()   # warm-up sink

    with tile.TileContext(nc) as tc:
        with (
            tc.tile_pool(name="resident", bufs=1) as rpool,
            tc.tile_pool(name="w1", bufs=3) as w1pool,
            tc.tile_pool(name="wd2", bufs=4) as wd2pool,
            tc.tile_pool(name="outp", bufs=4) as outpool,
            tc.tile_pool(name="psum", bufs=1, space="PSUM") as ppool,
        ):
            xt = rpool.tile([P, KD, cap], bf16)
            # a: A f-tile i in cols [0, capA) of plane i, B f-tile i in
            # cols [capA, cap) of plane i — packed, 22 planes not 44.
            at = rpool.tile([P, KFH, cap], bf16)

            def psum(kind, reg_idx, ci, cn, name=None):
                return ppool.tile([P, cn], f32,
                                  tag=f"{kind}{'AB'[reg_idx]}{ci}", name=name)

            # Warm-up: PE HAM clock ramp (1.2 -> 2.4 GHz) while first DMAs
            # are in flight; sized to end right when the first weight +
            # token DMAs complete so the clock never dips back to half
            # rate (an idle PE in the first ~30us downclocks to k=4 and
            # costs ~5us of half-rate work).
            NWARM = 23
            warm = rpool.tile([P, 512], bf16)
            nc.vector.memset(warm[:], 0.0)
            wps = ppool.tile([P, 512], f32, tag="warm", name="warm_ps")
            for i in range(NWARM):
                nc.tensor.matmul(wps[:], warm[:, :P], warm[:],
                                 start=(i == 0), stop=(i == NWARM - 1))
            wout = rpool.tile([P, 512], f32)
            nc.vector.tensor_copy(out=wout[:], in_=wps[:])
            nc.sync.dma_start(scr_d[:], wout[:])

            for fg in range(NFG):
                wgb = w1pool.tile([P, FG, KD, P], bf16, tag="wgb",
                                  name=f"wgb_{fg}")
                wub = w1pool.tile([P, FG, KD, P], bf16, tag="wub",
                                  name=f"wub_{fg}")
                if fg == 0:
                    # Queue order on the SWDGE ring decides arrival order
                    # (sync is NOT usable here: its first instruction gates
                    # on the warm-up, stalling anything queued behind it).
                    # The first f-tile's 16 k-passes consume ALL of xt, so
                    # xt streams right after the first 128-col wg slice, in
                    # 4-ktile pieces (fewer per-DMA issue latencies).
                    nc.gpsimd.dma_start(wgb[:, 0], wg_d[0][:, 0])
                    for k0 in range(0, KD, 4):
                        nc.gpsimd.dma_start(xt[:, k0:k0 + 4, :],
                                            xt_d[:, k0:k0 + 4, :])
                    nc.gpsimd.dma_start(wub[:, 0], wu_d[0][:, 0])
                    for s in range(1, FG):
                        nc.gpsimd.dma_start(wgb[:, s], wg_d[0][:, s])
                        nc.gpsimd.dma_start(wub[:, s], wu_d[0][:, s])
                else:
                    sh = FG // 2
                    nc.gpsimd.dma_start(wgb[:, :sh], wg_d[fg][:, :sh])
                    nc.gpsimd.dma_start(wgb[:, sh:], wg_d[fg][:, sh:])
                    nc.gpsimd.dma_start(wub[:, :sh], wu_d[fg][:, :sh])
                    nc.gpsimd.dma_start(wub[:, sh:], wu_d[fg][:, sh:])

                # ---- phase 1: gT/uT = W.T @ xT, aT = silu(gT)*uT ----
                for fs in range(FG):
                    ft = fg * FG + fs
                    f0, t0, cws = reg_of(ft)
                    ridx = 0 if ft < KFH else 1
                    if ft == 0:
                        # xt pieces land ~1.4us apart while this tile wants
                        # one 2-ko piece per ~0.5us: interleave both chunks
                        # per ko and pad each inter-piece hole with filler
                        # matmuls (no deps) so the PE stays busy and HAM
                        # never drops to half clock.
                        pgs = [psum("g", 0, ci, cn, name=f"pg0_{ci}")
                               for ci, cn in enumerate(cws)]
                        pus = [psum("u", 0, ci, cn, name=f"pu0_{ci}")
                               for ci, cn in enumerate(cws)]
                        for ko in range(KD):
                            c0 = t0
                            for ci, cn in enumerate(cws):
                                nc.tensor.matmul(
                                    pgs[ci][:], wgb[:, 0, ko, :],
                                    xt[:, ko, c0:c0 + cn],
                                    start=(ko == 0), stop=(ko == KD - 1))
                                c0 += cn
                            if ko % 4 == 3 and ko < KD - 4:
                                for _ in range(5):
                                    nc.tensor.matmul(wps[:], warm[:, :P],
                                                     warm[:], start=True,
                                                     stop=True)
                        for ko in range(KD):
                            c0 = t0
                            for ci, cn in enumerate(cws):
                                nc.tensor.matmul(
                                    pus[ci][:], wub[:, 0, ko, :],
                                    xt[:, ko, c0:c0 + cn],
                                    start=(ko == 0), stop=(ko == KD - 1))
                                c0 += cn
                        c0 = t0
                        for ci, cn in enumerate(cws):
                            a_sl = at[:, ft - f0, c0:c0 + cn]
                            nc.scalar.activation(
                                a_sl, pgs[ci][:],
                                mybir.ActivationFunctionType.Silu)
                            nc.vector.tensor_tensor(a_sl, a_sl, pus[ci][:],
                                                    mult)
                            c0 += cn
                        continue
                    c0 = t0
                    for ci, cn in enumerate(cws):
                        pg = psum("g", ridx, ci, cn, name=f"pg_{ft}_{ci}")
                        pu = psum("u", ridx, ci, cn, name=f"pu_{ft}_{ci}")
                        for ko in range(KD):
                            nc.tensor.matmul(
                                pg[:], wgb[:, fs, ko, :],
                                xt[:, ko, c0:c0 + cn],
                                start=(ko == 0), stop=(ko == KD - 1))
                        for ko in range(KD):
                            nc.tensor.matmul(
                                pu[:], wub[:, fs, ko, :],
                                xt[:, ko, c0:c0 + cn],
                                start=(ko == 0), stop=(ko == KD - 1))
                        a_sl = at[:, ft - f0, c0:c0 + cn]
                        nc.scalar.activation(
                            a_sl, pg[:], mybir.ActivationFunctionType.Silu)
                        nc.vector.tensor_tensor(a_sl, a_sl, pu[:], mult)
                        c0 += cn

            # ---- phase 2: dT_partial = Wd.T @ aT (combine on host) ----
            for dg in range(NDG):
                # accumulators: [ds][region][chunk]; ds=0 reuses g tags,
                # ds=1 the u tags (exact same widths)
                pds = [[[psum("gu"[ds], ridx, ci, cn,
                              name=f"pd_{dg}_{ds}_{ridx}_{ci}")
                         for ci, cn in enumerate(regs[ridx][2])]
                        for ridx in range(2)]
                       for ds in range(DG)]
                # last d-group runs B's blocks first so the kernel's very
                # last psum->sbuf copy + out DMA is the A part — a shorter
                # serial tail after the final matmul.  Accumulation flags
                # follow processing order (per-region counters), not fk.
                last = dg == NDG - 1
                fbseq = ([6, 7, 8, 9, 10, 5, 0, 1, 2, 3, 4] if last
                         else list(range(NFB)))
                done = [0, 0]
                for fb in fbseq:
                    wdb = wd2pool.tile([P, KO2, P * DG], bf16, tag="wdb",
                                       name=f"wdb_{dg}_{fb}")
                    kh2 = KO2 // 2
                    nc.gpsimd.dma_start(wdb[:, :kh2, :],
                                        wd_d[dg, fb][:, :kh2, :])
                    nc.gpsimd.dma_start(wdb[:, kh2:, :],
                                        wd_d[dg, fb][:, kh2:, :])
                    for ko in range(KO2):
                        fk = fb * KO2 + ko
                        f0, t0, cws = reg_of(fk)
                        ridx = 0 if fk < KFH else 1
                        for ds in range(DG):
                            c0 = t0
                            for ci, cn in enumerate(cws):
                                nc.tensor.matmul(
                                    pds[ds][ridx][ci][:],
                                    wdb[:, ko, ds * P:(ds + 1) * P],
                                    at[:, fk - f0, c0:c0 + cn],
                                    start=(done[ridx] == 0),
                                    stop=(done[ridx] == KFH - 1))
                                c0 += cn
                        done[ridx] += 1
                for ds in range(DG):
                    ot = outpool.tile([P, cap], bf16, tag="ot")
                    dt_idx = dg * DG + ds
                    orow = out_d[dt_idx * P:(dt_idx + 1) * P, :]
                    # emit each region's slice as soon as its accumulation
                    # is done (the other region's matmuls overlap the copy)
                    rseq = (1, 0) if last else (0, 1)
                    for ridx in rseq:
                        f0, t0, cws = regs[ridx]
                        c0 = t0
                        for ci, cn in enumerate(cws):
                            nc.vector.tensor_copy(out=ot[:, c0:c0 + cn],
                                                  in_=pds[ds][ridx][ci][:])
                            c0 += cn
                        nc.sync.dma_start(orow[:, t0:c0], ot[:, t0:c0])

    nc.compile()
    return nc


def _swizzle_w1(w):
    """[D, F] -> [NFG, P, FG, KD, P] s-major: every 128-col s-slice of a
    block is contiguous per partition (128-descriptor DMAs)."""
    return np.ascontiguousarray(
        w.reshape(KD, P, NFG, FG, P).transpose(2, 1, 3, 0, 4)).astype(BF16)


def _swizzle_wd(w):
    """[F, D] -> [NDG, NFB, P, KO2, P*DG] block-major."""
    return np.ascontiguousarray(
        w.reshape(NFB, KO2, P, NDG, P * DG).transpose(3, 0, 2, 1, 4)).astype(BF16)


def kernel(x, gate_tensor, Wg, Wu, Wd):
    global LAST_RESULT
    from concourse.bass_interp import get_hw_module
    from concourse.bass_utils import run_bass_kernel_spmd

    x = np.ascontiguousarray(np.asarray(x, dtype=np.float32))
    gate_tensor = np.asarray(gate_tensor, dtype=np.float32)
    Wg = np.asarray(Wg, dtype=np.float32)
    Wu = np.asarray(Wu, dtype=np.float32)
    Wd = np.asarray(Wd, dtype=np.float32)

    # ---- router (replicated; tiny: T*D*E flops) ----
    logits = x @ gate_tensor                      # [T, E] fp32
    m = logits.max(axis=-1, keepdims=True)
    p = np.exp(logits - m, dtype=np.float32)
    p /= p.sum(axis=-1, keepdims=True)
    topi = np.argsort(-p, axis=-1, kind="stable")[:, :TOPK]      # [T, K]
    topw = np.take_along_axis(p, topi, axis=-1)
    topw = topw / (topw.sum(axis=-1, keepdims=True) + 1e-20)

    idx = []          # tokens routed to each expert
    wts = []          # their combine weights
    for e in range(E):
        sel = (topi == e)                         # [T, K]; <=1 True per row
        idx.append(np.nonzero(sel.any(axis=-1))[0])
        wts.append(topw[sel].astype(np.float32))  # row-major == token order

    # ---- pairing: i-th heaviest with i-th lightest ----
    counts = np.array([len(t) for t in idx])
    order = np.argsort(-counts, kind="stable")
    pairs = [(int(order[i]), int(order[E - 1 - i])) for i in range(E // 2)]
    capA = max(2, (int(counts[order[:E // 2]].max()) + 1) // 2 * 2)
    capB = max(2, (int(counts[order[E // 2:]].max()) + 1) // 2 * 2)
    cap = capA + capB

    key = (capA, capB)
    if key not in _COMPILED:
        _COMPILED[key] = _build(capA, capB)
    nc = _COMPILED[key]

    # ---- dispatch: per-core inputs (pre-swizzled, bf16) ----
    halfF = F // 2
    in_maps = []
    for p_i, (eh, el) in enumerate(pairs):
        nh, nl = len(idx[eh]), len(idx[el])
        xt = np.zeros((P, KD, cap), dtype=BF16)
        xt[:, :, :nh] = (x[idx[eh]].T.reshape(KD, P, nh)
                         .transpose(1, 0, 2).astype(BF16))
        xt[:, :, capA:capA + nl] = (x[idx[el]].T.reshape(KD, P, nl)
                                    .transpose(1, 0, 2).astype(BF16))
        for h in range(2):
            hsl = slice(h * halfF, (h + 1) * halfF)
            wg = _swizzle_w1(np.concatenate(
                [Wg[eh][:, hsl], Wg[el][:, hsl]], axis=1))
            wu = _swizzle_w1(np.concatenate(
                [Wu[eh][:, hsl], Wu[el][:, hsl]], axis=1))
            wd = _swizzle_wd(np.concatenate(
                [Wd[eh][hsl, :], Wd[el][hsl, :]], axis=0))
            in_maps.append({"xt": xt, "wg": wg, "wu": wu, "wd": wd})

    trace = bool(int(os.environ.get("KERNEL_TRACE", "0")))
    old_m = nc.m
    nc.m = get_hw_module(nc.m)
    try:
        try:
            res = run_bass_kernel_spmd(nc, in_maps, core_ids=list(range(E)),
                                       trace=trace)
        except (ImportError, ModuleNotFoundError):
            # tracing requested but this image lacks the axon NTFF profile
            # hook -- rerun without tracing
            os.environ["BASS_NEVER_TRACE"] = "1"
            res = run_bass_kernel_spmd(nc, in_maps, core_ids=list(range(E)),
                                       trace=False)
    finally:
        nc.m = old_m
    LAST_RESULT = res

    # ---- combine: add F-half partials, weight, scatter-add ----
    out = np.zeros((T, D), dtype=np.float32)
    for p_i, (eh, el) in enumerate(pairs):
        nh, nl = len(idx[eh]), len(idx[el])
        d = (res.results[2 * p_i]["out_t"].astype(np.float32)
             + res.results[2 * p_i + 1]["out_t"].astype(np.float32))
        out[idx[eh]] += wts[eh][:, None] * d[:, :nh].T
        out[idx[el]] += wts[el][:, None] * d[:, capA:capA + nl].T
    return out
